# revision 1
# baseline (speedup 1.0000x reference)
"""Causal self-attention (B=4, S=2048, E=1024, H=16, D=64) on 8 TRN2 cores.

Sharding: core c handles batch b = c//2 and heads [8*(c%2), 8*(c%2)+8).
Each core computes qkv for its 8 heads, full attention for them, and a
partial output projection; the host sums the two partial projections per
batch (the "all-reduce after proj" done host-side).

Device-side layout (per core):
  - QT, KT kept transposed [d_local, s]  (d on partitions) so that
    scores^T[k, q] = KT_chunk.T @ QT  needs no transposes anywhere.
  - V kept natural [s, d_local] with a ones-column appended per head, so
    the AV matmul  U = V'.T @ expS^T  yields both Y^T (rows 0:64) and the
    softmax denominator (row 64) in one accumulation group.
  - softmax: exp on ScalarE (1/sqrt(D) scale fused into the activation),
    causal masking via one host-precomputed [128, 128] 0/1 triangle applied
    to the diagonal 128-column block of each diagonal k-chunk (the fully
    masked columns are excluded from the AV matmul instead), denominator
    reciprocal on VectorE (partition-parallel, batched per head pair),
    broadcast across partitions on GpSimdE (kernel tail: via a rank-1
    matmul on the then-idle PE).
  - The two heads of a slab run as interleaved independent dependency
    chains; normalization of pair j-1 is pipelined into pair j's window.
"""

import numpy as np
import ml_dtypes

import concourse.bass as bass
import concourse.bacc as bacc
import concourse.mybir as mybir
import concourse.tile as tile
from concourse.bass_utils import run_bass_kernel_spmd


BF16 = mybir.dt.bfloat16
F32 = mybir.dt.float32
AF = mybir.ActivationFunctionType

B, S, E = 4, 2048, 1024
H, D = 16, 64
HL = 8                # heads per core
DL = HL * D           # 512 local head dims
NSLAB = HL // 2       # 4 partition slabs of 2 heads (128 dims) each
KCH = E // 128        # 8 contraction chunks for the qkv matmuls
NQ = S // 512         # 4 query blocks of 512
NK = S // 128         # 16 key chunks of 128
NM = S // 128         # 16 output row chunks
G = 2                 # k-chunks per exp group (2 psum banks per scores tile)

_CACHE: dict = {}


def _emit(nc: bass.Bass, tc, ap):
    xt, wq, wk, wv, wp = ap["xt"], ap["wq"], ap["wk"], ap["wv"], ap["wp"]
    bq, bk, bv, bp, mk, out = ap["bq"], ap["bk"], ap["bv"], ap["bp"], ap["mk"], ap["out"]

    import contextlib
    ctx = contextlib.ExitStack()
    with ctx:
        const = ctx.enter_context(tc.tile_pool(name="const", bufs=1))
        exp_pool = ctx.enter_context(tc.tile_pool(name="exp", bufs=8))
        out_pool = ctx.enter_context(tc.tile_pool(name="outp", bufs=4))
        cl_pool = ctx.enter_context(tc.tile_pool(name="clp", bufs=4))
        r_pool = ctx.enter_context(tc.tile_pool(name="rp", bufs=2))
        rr_pool = ctx.enter_context(tc.tile_pool(name="rrp", bufs=3))
        rb_pool = ctx.enter_context(tc.tile_pool(name="rbp", bufs=4))
        yb_pool = ctx.enter_context(tc.tile_pool(name="ybp", bufs=16))
        acc_ps = ctx.enter_context(tc.tile_pool(name="acc", bufs=2, space="PSUM"))
        sc_ps = ctx.enter_context(tc.tile_pool(name="scps", bufs=3, space="PSUM"))

        # ---- persistent SBUF tensors + input DMA ----
        # xt on the sync queue, weights on the gpsimd queue: two DMA paths
        # fill in parallel and the first QT matmuls start sooner.
        xt_sb = const.tile([128, KCH, S], BF16, name="xt_sb")
        wq_sb = const.tile([128, KCH, DL], BF16, name="wq_sb")
        wk_sb = const.tile([128, KCH, DL], BF16, name="wk_sb")
        wv_sb = const.tile([128, KCH, DL], BF16, name="wv_sb")
        for k in range(KCH):
            nc.sync.dma_start(xt_sb[:, k, 0:1024], xt[k * 128:(k + 1) * 128, 0:1024])
            nc.sync.dma_start(xt_sb[:, k, 1024:2048], xt[k * 128:(k + 1) * 128, 1024:2048])
            nc.gpsimd.dma_start(wq_sb[:, k, :], wq[k * 128:(k + 1) * 128, :])
            nc.gpsimd.dma_start(wk_sb[:, k, :], wk[k * 128:(k + 1) * 128, :])
        for k in range(KCH):
            nc.gpsimd.dma_start(wv_sb[:, k, :], wv[k * 128:(k + 1) * 128, :])
        wp_sb = const.tile([128, NSLAB, E], BF16, name="wp_sb")
        for j in range(NSLAB):
            nc.gpsimd.dma_start(wp_sb[:, j, :], wp[j * 128:(j + 1) * 128, :])
        bq_sb = const.tile([128, NSLAB], F32, name="bq_sb")
        nc.sync.dma_start(bq_sb[:, :], bq[:, :])
        bk_sb = const.tile([128, NSLAB], F32, name="bk_sb")
        nc.sync.dma_start(bk_sb[:, :], bk[:, :])
        bv_sb = const.tile([128, DL], F32, name="bv_sb")
        nc.gpsimd.dma_start(bv_sb[:, :], bv[:, :])
        bp_sb = const.tile([128, E], F32, name="bp_sb")
        nc.gpsimd.dma_start(bp_sb[:, :], bp[:, :])
        mk_sb = const.tile([128, 128], BF16, name="mk_sb")
        nc.gpsimd.dma_start(mk_sb[:, :], mk[:, :])

        qt_sb = const.tile([128, NSLAB, S], BF16, name="qt_sb")
        kt_sb = const.tile([128, NSLAB, S], BF16, name="kt_sb")
        v_sb = const.tile([128, NK, HL, D + 1], BF16, name="v_sb")
        yt_sb = const.tile([128, NSLAB, S], BF16, name="yt_sb")

        # ---- phase 1: qkv projections ----
        # QT[d, s] / KT[d, s]: slab j holds heads (2j, 2j+1). V for all heads.
        # Only V and slab 0's QT/KT are computed up front; later slabs'
        # QT/KT are emitted inside the previous pair's attention window so
        # the PE has dense work while ACT runs exp (keeps HAM warm).
        def qtkt_slab(j):
            # all 4 query-column blocks accumulate together so the 4 matmuls
            # per k-chunk share one stationary weight load (3 psum banks are
            # borrowed from the idle scores pool during this phase)
            for w_sb, b_sb, dst in ((wq_sb, bq_sb, qt_sb), (wk_sb, bk_sb, kt_sb)):
                pa = sc_ps.tile([128, G, 512], F32, name="pa", tag="sc")
                pa2 = sc_ps.tile([128, G, 512], F32, name="pa2", tag="sc")
                slabs = [pa[:, 0, :], pa[:, 1, :], pa2[:, 0, :], pa2[:, 1, :]]
                for k in range(KCH):
                    for sj in range(NQ):
                        nc.tensor.matmul(
                            slabs[sj],
                            lhsT=w_sb[:, k, j * 128:(j + 1) * 128],
                            rhs=xt_sb[:, k, sj * 512:(sj + 1) * 512],
                            start=(k == 0), stop=(k == KCH - 1),
                        )
                for sj in range(NQ):
                    nc.vector.tensor_scalar_add(
                        dst[:, j, sj * 512:(sj + 1) * 512], slabs[sj], b_sb[:, j:j + 1]
                    )

        def v_all():
            for sc in range(NK):
                ps = acc_ps.tile([128, 512], F32, name="ps", tag="acc")
                for k in range(KCH):
                    nc.tensor.matmul(
                        ps[:, :],
                        lhsT=xt_sb[:, k, sc * 128:(sc + 1) * 128],
                        rhs=wv_sb[:, k, :],
                        start=(k == 0), stop=(k == KCH - 1),
                    )
                nc.vector.tensor_add(
                    v_sb[:, sc, :, 0:D],
                    ps.rearrange("p (h d) -> p h d", h=HL),
                    bv_sb.rearrange("p (h d) -> p h d", h=HL),
                )
                nc.vector.memset(v_sb[:, sc, :, D:D + 1], 1.0)

        qtkt_slab(0)
        v_all()

        # ---- phase 2: attention, head h; normalization of head h-1 pipelined
        # into head h's attention window so the PE never waits on it.
        # Denominators collect into [128, 512] (row at partition 32*qi) and are
        # inverted with one partition-parallel DVE reciprocal per head.
        ones_sb = const.tile([128, D], F32, name="ones_sb")
        nc.vector.memset(ones_sb[:, :], 1.0)
        pair_state = {}

        def attn_pair(j, filler=()):
            filler = list(filler)
            # the two heads (base partitions 0 and 64 of slab j) run as two
            # independent dependency chains interleaved group-by-group, so the
            # PE always has a sibling chain to work on while ACT runs exp.
            cls, ybs = [], [{}, {}]
            for i in range(2):
                cl_t = cl_pool.tile([128, 512], F32, name="cl_t", tag="cl")
                nc.vector.memset(cl_t[:, :], 1.0)
                cls.append(cl_t)
            for qi in range(NQ):
                nk = 4 * (qi + 1)  # causal: k chunks 0..nk-1 needed
                us = [acc_ps.tile([128, 512], F32, name="u_ps", tag="acc")
                      for _ in range(2)]
                for g0 in range(0, nk, G):
                    gl = min(G, nk - g0)
                    # scores for both heads, interleaved per chunk: head 0 uses
                    # PE rows 0:64, head 1 rows 64:128 — consecutive row-disjoint
                    # matmuls execute concurrently in the array (row tiling).
                    scs, exs = [], []
                    for i in range(2):
                        scs.append(sc_ps.tile([128, G, 512], F32, name="sc_t", tag="sc"))
                        exs.append(exp_pool.tile([128, G, 512], BF16, name="ex_t", tag="ex"))
                    for ci in range(gl):
                        kc = g0 + ci
                        for i in range(2):
                            nc.tensor.matmul(
                                scs[i][:, ci, :],
                                lhsT=kt_sb[64 * i:64 * i + 64, j, kc * 128:(kc + 1) * 128],
                                rhs=qt_sb[64 * i:64 * i + 64, j, qi * 512:(qi + 1) * 512],
                                start=True, stop=True,
                            )
                    for i in range(2):
                        nc.scalar.activation(
                            exs[i][:, 0:gl, :], scs[i][:, 0:gl, :], AF.Exp,
                            scale=float(D) ** -0.5,
                        )
                    for i in range(2):
                        for ci in range(gl):
                            dc = (g0 + ci) - 4 * qi
                            if 0 <= dc <= 3:
                                # only the 128-wide triangle block needs the
                                # mask; columns < 128*dc are skipped by the AV
                                sl = slice(128 * dc, 128 * dc + 128)
                                nc.vector.tensor_mul(
                                    exs[i][:, ci, sl], exs[i][:, ci, sl], mk_sb[:, :]
                                )
                    for i in range(2):
                        h = 2 * j + i
                        for ci in range(gl):
                            kc = g0 + ci
                            dc = kc - 4 * qi
                            c0 = 128 * dc if 0 <= dc <= 3 else 0
                            nc.tensor.matmul(
                                us[i][0:D + 1, c0:512],
                                lhsT=v_sb[:, kc, h, :],
                                rhs=exs[i][:, ci, c0:512],
                                start=(kc == 0), stop=(kc == nk - 1),
                            )
                for i in range(2):
                    nc.vector.tensor_copy(
                        cls[i][32 * qi:32 * qi + 1, :], us[i][D:D + 1, :]
                    )
                    yb_t = yb_pool.tile([64, 512], BF16, name="yb_t", tag="yb")
                    nc.vector.tensor_copy(yb_t[:, :], us[i][0:D, :])
                    ybs[i][qi] = yb_t
                for _ in range(2):
                    if filler:
                        filler.pop(0)()
            while filler:
                filler.pop(0)()
            pair_state[j] = (cls, ybs)

        def norm_pair(j, last=False):
            cls, ybs = pair_state.pop(j)
            for i in range(2):
                r4_t = r_pool.tile([128, 512], F32, name="r4_t", tag="r")
                # split so the scheduler can interleave mask-mults between
                # pieces instead of stalling the DVE FIFO for 3.3us
                for rc in range(4):
                    nc.vector.reciprocal(
                        r4_t[:, rc * 128:(rc + 1) * 128],
                        cls[i][:, rc * 128:(rc + 1) * 128],
                    )
                for qi in range(NQ):
                    ysl = yt_sb[64 * i:64 * i + 64, j, qi * 512:(qi + 1) * 512]
                    if last and qi == 3:
                        # base partition 96 is illegal for matmul operands:
                        # hop the r row to partition 0 first, then PE-bcast
                        rr_t = rr_pool.tile([1, 512], F32, name="rr_t", tag="rr")
                        nc.sync.dma_start(rr_t[:, :], r4_t[96:97, :])
                        bc_t = sc_ps.tile([128, G, 512], F32, name="bc_t", tag="sc")
                        nc.tensor.matmul(
                            bc_t[0:D, 0, :],
                            lhsT=ones_sb[0:1, :],
                            rhs=rr_t[:, :],
                            start=True, stop=True,
                        )
                        nc.vector.tensor_mul(ysl, ybs[i][qi][:, :], bc_t[0:D, 0, :])
                    elif last:  # matmul operands must sit at base 0/32/64
                        # PE is idle at the kernel tail: broadcast r via a
                        # rank-1 matmul into a free scores bank instead of the
                        # DMA+gpsimd chain.
                        bc_t = sc_ps.tile([128, G, 512], F32, name="bc_t", tag="sc")
                        nc.tensor.matmul(
                            bc_t[0:D, 0, :],
                            lhsT=ones_sb[32 * qi:32 * qi + 1, :],
                            rhs=r4_t[32 * qi:32 * qi + 1, :],
                            start=True, stop=True,
                        )
                        nc.vector.tensor_mul(ysl, ybs[i][qi][:, :], bc_t[0:D, 0, :])
                    else:
                        rr_t = rr_pool.tile([1, 512], F32, name="rr_t", tag="rr")
                        nc.sync.dma_start(rr_t[:, :], r4_t[32 * qi:32 * qi + 1, :])
                        rb_t = rb_pool.tile([64, 512], F32, name="rb_t", tag="rb")
                        nc.gpsimd.partition_broadcast(rb_t[:, :], rr_t[:, :])
                        nc.vector.tensor_mul(ysl, ybs[i][qi][:, :], rb_t[:, :])

        for j in range(1, NSLAB):
            qtkt_slab(j)
        for j in range(NSLAB):
            attn_pair(j)
            if j > 0:
                norm_pair(j - 1)
        norm_pair(NSLAB - 1, last=True)

        # ---- phase 3: output projection (partial; host sums core pairs) ----
        # both 512-wide output blocks accumulate together so the two matmuls
        # per slab share one stationary (yt chunk) weight load
        for m in range(NM):
            ps0 = acc_ps.tile([128, 512], F32, name="ps0", tag="acc")
            ps1 = sc_ps.tile([128, G, 512], F32, name="ps1", tag="sc")
            for j in range(NSLAB):
                nc.tensor.matmul(
                    ps0[:, :],
                    lhsT=yt_sb[:, j, m * 128:(m + 1) * 128],
                    rhs=wp_sb[:, j, 0:512],
                    start=(j == 0), stop=(j == NSLAB - 1),
                )
                nc.tensor.matmul(
                    ps1[:, 0, :],
                    lhsT=yt_sb[:, j, m * 128:(m + 1) * 128],
                    rhs=wp_sb[:, j, 512:1024],
                    start=(j == 0), stop=(j == NSLAB - 1),
                )
            for n, ps in ((0, ps0[:, :]), (1, ps1[:, 0, :])):
                o_t = out_pool.tile([128, 512], F32, name="o_t", tag="ot")
                nc.vector.tensor_add(o_t[:, :], ps, bp_sb[:, n * 512:(n + 1) * 512])
                nc.sync.dma_start(out[m * 128:(m + 1) * 128, n * 512:(n + 1) * 512], o_t[:, :])


def build():
    if "nc" in _CACHE:
        return _CACHE["nc"]
    nc = bacc.Bacc("TRN2", debug=False)
    ap = {
        "xt": nc.dram_tensor("xt", [E, S], BF16, kind="ExternalInput").ap(),
        "wq": nc.dram_tensor("wq", [E, DL], BF16, kind="ExternalInput").ap(),
        "wk": nc.dram_tensor("wk", [E, DL], BF16, kind="ExternalInput").ap(),
        "wv": nc.dram_tensor("wv", [E, DL], BF16, kind="ExternalInput").ap(),
        "wp": nc.dram_tensor("wp", [DL, E], BF16, kind="ExternalInput").ap(),
        "bq": nc.dram_tensor("bq", [128, NSLAB], F32, kind="ExternalInput").ap(),
        "bk": nc.dram_tensor("bk", [128, NSLAB], F32, kind="ExternalInput").ap(),
        "bv": nc.dram_tensor("bv", [128, DL], F32, kind="ExternalInput").ap(),
        "bp": nc.dram_tensor("bp", [128, E], F32, kind="ExternalInput").ap(),
        "mk": nc.dram_tensor("mk", [128, 128], BF16, kind="ExternalInput").ap(),
        "out": nc.dram_tensor("out", [S, E], F32, kind="ExternalOutput").ap(),
    }
    with tile.TileContext(nc) as tc:
        _emit(nc, tc, ap)
    nc.compile()
    _CACHE["nc"] = nc
    return nc


def make_in_maps(x, w_qkv, b_qkv, w_proj, b_proj):
    """Host-side sharding: one input map per core."""
    bf = ml_dtypes.bfloat16
    in_maps = []
    for c in range(8):
        b, half = c // 2, c % 2
        hbase = half * HL
        dsl = slice(hbase * D, hbase * D + DL)
        xt = np.ascontiguousarray(x[b].T).astype(bf)
        wqs = np.ascontiguousarray(w_qkv[:, 0 * E:1 * E][:, dsl]).astype(bf)
        wks = np.ascontiguousarray(w_qkv[:, 1 * E:2 * E][:, dsl]).astype(bf)
        wvs = np.ascontiguousarray(w_qkv[:, 2 * E:3 * E][:, dsl]).astype(bf)
        wps = np.ascontiguousarray(w_proj[dsl, :]).astype(bf)
        bqs = np.ascontiguousarray(
            b_qkv[0 * E:1 * E][dsl].reshape(NSLAB, 128).T).astype(np.float32)
        bks = np.ascontiguousarray(
            b_qkv[1 * E:2 * E][dsl].reshape(NSLAB, 128).T).astype(np.float32)
        bvs = np.broadcast_to(b_qkv[2 * E:3 * E][dsl], (128, DL)).astype(np.float32)
        # both cores of a batch pair add bp and the host sums them: halve it
        bps = np.broadcast_to(b_proj * 0.5, (128, E)).astype(np.float32)
        kk = np.arange(128)[:, None]
        qq = np.arange(128)[None, :]
        mks = (kk <= qq).astype(bf)
        in_maps.append({
            "xt": xt, "wq": wqs, "wk": wks, "wv": wvs, "wp": wps,
            "bq": bqs, "bk": bks, "bv": np.ascontiguousarray(bvs),
            "bp": np.ascontiguousarray(bps), "mk": np.ascontiguousarray(mks),
        })
    return in_maps


def kernel(x, w_qkv, b_qkv, w_proj, b_proj, _trace=False):
    x = np.asarray(x, np.float32)
    w_qkv = np.asarray(w_qkv, np.float32)
    b_qkv = np.asarray(b_qkv, np.float32)
    w_proj = np.asarray(w_proj, np.float32)
    b_proj = np.asarray(b_proj, np.float32)
    nc = build()
    in_maps = make_in_maps(x, w_qkv, b_qkv, w_proj, b_proj)
    res = run_bass_kernel_spmd(nc, in_maps, core_ids=list(range(8)), trace=_trace)
    _CACHE["last_results"] = res
    out = np.empty((B, S, E), dtype=np.float32)
    for b in range(B):
        out[b] = res.results[2 * b]["out"] + res.results[2 * b + 1]["out"]
    return out

